# revision 7
# baseline (speedup 1.0000x reference)
"""Multi-head self-attention (B=2, S=2048, D=1024, H=16, causal) on 8 NeuronCores.

Sharding: 32 (batch, head) instances -> 4 heads of one batch per core
(cores 0-3: batch 0, cores 4-7: batch 1; core c owns heads 4*(c%4) .. +3).
Wq/Wk/Wv are split by rows (head dims), Wo by columns; each core computes a
partial y[b] = attn_out_heads @ Wo_cols.T and the host sums the 4 partials
per batch at gather time (tensor-parallel reduce).

Per-core kernel (all matmuls float32r, PE-only, no on-device transposes):
  QT[256,2048] = wqT.T @ xT        (head-pair tiles: rows 0-63 / 64-127)
  KT likewise; V[2048,256] natural, augmented with a ones column per head.
  Scores computed transposed, blockwise [k-tile 128, q-chunk 512]:
      S^T = KT_h.T @ QT_h   -- two heads row-packed (contraction d=64 at
      partition bases 0 / 64 -> concurrent PE row groups)
  P = exp(S^T / 8) on ScalarE (psum pairs [128,1024] to amortize overhead),
  causal masking only on diagonal blocks via precomputed 0/1 mask multiply.
  AV: lhsT = [V_h | 1] [k,65], rhs = P [k,512] -> psum [65,512] accumulated
      over k-tiles = unnormalized out^T (rows 0-63) + softmax denominators
      (row 64). Normalize columns via reciprocal + gpsimd partition
      broadcast + DVE multiply, assembling out_headsT [256, 2048].
  y = out_headsT.T @ woT -> [2048, 1024] partial, DMA'd out.
"""
import os
import sys

sys.path.insert(0, "/opt/trn_rl_repo")

import numpy as np

import concourse.bass as bass  # noqa: F401  (import registers engine types)
import concourse.mybir as mybir
from concourse import bacc
from concourse.tile import TileContext
from concourse.bass_utils import run_bass_kernel_spmd

B, S, D = 2, 2048, 1024
H, HD = 16, 64
NCORES = 8
HPC = 4            # heads per core
SC = 512           # q-chunk width
KT_W = 128         # k-tile width
NQC = S // SC      # 4 q-chunks
NKT = S // KT_W    # 16 k-tiles
F32R = mybir.dt.float32r
F32 = mybir.dt.float32
ATTN_SCALE = 1.0 / np.sqrt(HD)

_CACHE = {}


def _build():
    nc = bacc.Bacc("TRN2", target_bir_lowering=False, debug=False, num_devices=NCORES)

    xT_d = nc.declare_dram_parameter("xT", [D, S], F32R, isOutput=False)
    wqT_d = nc.declare_dram_parameter("wqT", [D, 256], F32R, isOutput=False)
    wkT_d = nc.declare_dram_parameter("wkT", [D, 256], F32R, isOutput=False)
    wvT_d = nc.declare_dram_parameter("wvT", [D, 256], F32R, isOutput=False)
    woT_d = nc.declare_dram_parameter("woT", [256, D], F32R, isOutput=False)
    mask_d = nc.declare_dram_parameter("mask", [4, 128, SC], F32R, isOutput=False)
    ones_d = nc.declare_dram_parameter("ones", [128, HPC], F32R, isOutput=False)
    y_d = nc.declare_dram_parameter("y", [S, D], F32, isOutput=True)

    with TileContext(nc) as tc:
        with (
            tc.tile_pool(name="static", bufs=1) as st,
            tc.tile_pool(name="ppool", bufs=2) as ppool,
            tc.tile_pool(name="rbpool", bufs=2) as rbpool,
            tc.tile_pool(name="recpool", bufs=1) as recpool,
            tc.tile_pool(name="psA", bufs=2, space="PSUM") as psA,
            tc.tile_pool(name="psS", bufs=1, space="PSUM") as psS,
            tc.tile_pool(name="psV", bufs=1, space="PSUM") as psV,
        ):
            # ---- input DMAs ----
            xT = [st.tile([128, S], F32R, name=f"xT{k}", tag=f"xT{k}") for k in range(8)]
            for k in range(8):
                nc.sync.dma_start(out=xT[k][:], in_=xT_d[128 * k : 128 * k + 128, :])
            wq = st.tile([128, 2048], F32R, name="wq", tag="wq")
            wk = st.tile([128, 2048], F32R, name="wk", tag="wk")
            wv = st.tile([128, 2048], F32R, name="wv", tag="wv")
            for wt, wd in ((wq, wqT_d), (wk, wkT_d), (wv, wvT_d)):
                for k in range(8):
                    nc.sync.dma_start(
                        out=wt[:, 256 * k : 256 * k + 256],
                        in_=wd[128 * k : 128 * k + 128, :],
                    )
            wo = st.tile([128, 2048], F32R, name="wo", tag="wo")
            for cc in range(2):
                nc.sync.dma_start(
                    out=wo[:, 1024 * cc : 1024 * cc + 1024],
                    in_=woT_d[128 * cc : 128 * cc + 128, :],
                )
            mask = st.tile([128, 4 * SC], F32R, name="mask", tag="mask")
            for t in range(4):
                nc.sync.dma_start(out=mask[:, SC * t : SC * t + SC], in_=mask_d[t])

            # ---- projections ----
            QT = [st.tile([128, S], F32R, name=f"QT{m}", tag=f"QT{m}") for m in range(2)]
            KT = [st.tile([128, S], F32R, name=f"KT{m}", tag=f"KT{m}") for m in range(2)]
            for dst, w in ((QT, wq), (KT, wk)):
                for m in range(2):
                    for n in range(NQC):
                        acc = psA.tile([128, SC], F32, name="acc", tag="acc")
                        for k in range(8):
                            nc.tensor.matmul(
                                acc[:],
                                w[:, 256 * k + 128 * m : 256 * k + 128 * m + 128],
                                xT[k][:, SC * n : SC * n + SC],
                                start=(k == 0),
                                stop=(k == 7),
                            )
                        nc.vector.tensor_copy(dst[m][:, SC * n : SC * n + SC], acc[:])

            va = [
                st.tile([128, 65 * HPC], F32R, name=f"va{i}", tag=f"va{i}")
                for i in range(NKT)
            ]
            for i in range(NKT):
                accv = psA.tile([128, 256], F32, name="accv", tag="acc")
                for k in range(8):
                    nc.tensor.matmul(
                        accv[:],
                        xT[k][:, 128 * i : 128 * i + 128],
                        wv[:, 256 * k : 256 * k + 256],
                        start=(k == 0),
                        stop=(k == 7),
                    )
                for h in range(HPC):
                    nc.vector.tensor_copy(
                        va[i][:, 65 * h : 65 * h + 64], accv[:, 64 * h : 64 * h + 64]
                    )
                ones_ap = va[i].rearrange("p (h c) -> p h c", c=65)[:, :, 64]
                nc.sync.dma_start(out=ones_ap, in_=ones_d[:])

            # ---- attention ----
            outT = [
                st.tile([128, S], F32R, name=f"outT{m}", tag=f"outT{m}") for m in range(2)
            ]
            for hp in range(2):
                for jq in range(NQC):
                    nkt = 4 * jq + 4  # causal: k-tiles 0 .. 4*jq+3
                    av = [
                        psV.tile([65, SC], F32, name=f"av{u}", tag=f"av{u}")
                        for u in range(2)
                    ]
                    for kp in range((nkt + 1) // 2):
                        sp = [
                            psS.tile([128, 1024], F32, name=f"sp{u}", tag=f"sp{u}")
                            for u in range(2)
                        ]
                        kts = [2 * kp, 2 * kp + 1]  # nkt is always even
                        for u, base in enumerate((0, 64)):
                            for j, kt in enumerate(kts):
                                nc.tensor.matmul(
                                    sp[u][:, 512 * j : 512 * j + 512],
                                    KT[hp][base : base + 64, 128 * kt : 128 * kt + 128],
                                    QT[hp][base : base + 64, SC * jq : SC * jq + SC],
                                    start=True,
                                    stop=True,
                                )
                        pt = [
                            ppool.tile([128, 1024], F32R, name=f"pt{u}", tag=f"pt{u}")
                            for u in range(2)
                        ]
                        for u in range(2):
                            nc.scalar.activation(
                                pt[u][:],
                                sp[u][:],
                                mybir.ActivationFunctionType.Exp,
                                scale=float(ATTN_SCALE),
                            )
                        for j, kt in enumerate(kts):
                            t = kt - 4 * jq
                            if t >= 0:  # diagonal block: causal mask
                                for u in range(2):
                                    nc.vector.tensor_mul(
                                        pt[u][:, 512 * j : 512 * j + 512],
                                        pt[u][:, 512 * j : 512 * j + 512],
                                        mask[:, SC * t : SC * t + SC],
                                    )
                        for j, kt in enumerate(kts):
                            for u in range(2):
                                h = 2 * hp + u
                                nc.tensor.matmul(
                                    av[u][:],
                                    va[kt][:, 65 * h : 65 * h + 65],
                                    pt[u][:, 512 * j : 512 * j + 512],
                                    start=(kt == 0),
                                    stop=(kt == nkt - 1),
                                )
                    # normalize: columns /= denominators (row 64)
                    for u in range(2):
                        rec = recpool.tile([1, SC], F32, name="rec", tag="rec")
                        nc.vector.reciprocal(rec[:], av[u][64:65, :])
                        rb = rbpool.tile([64, SC], F32, name="rb", tag="rb")
                        nc.gpsimd.partition_broadcast(rb[:], rec[:])
                        nc.vector.tensor_mul(
                            outT[hp][64 * u : 64 * u + 64, SC * jq : SC * jq + SC],
                            av[u][0:64, :],
                            rb[:],
                        )

            # ---- output projection ----
            for i in range(NKT):  # 16 s-tiles of 128
                for n in range(2):  # dout chunks of 512
                    yp = psA.tile([128, 512], F32, name="yp", tag="acc")
                    for cc in range(2):
                        nc.tensor.matmul(
                            yp[:],
                            outT[cc][:, 128 * i : 128 * i + 128],
                            wo[:, 1024 * cc + 512 * n : 1024 * cc + 512 * n + 512],
                            start=(cc == 0),
                            stop=(cc == 1),
                        )
                    ys = ppool.tile([128, 512], F32, name="ys", tag="pt0")
                    nc.vector.tensor_copy(ys[:], yp[:])
                    nc.sync.dma_start(
                        out=y_d[128 * i : 128 * i + 128, 512 * n : 512 * n + 512],
                        in_=ys[:],
                    )

    nc.compile()
    return nc


def _masks_np():
    m = np.zeros((4, 128, SC), dtype=np.float32)
    qq = np.arange(SC)[None, :]
    kk = np.arange(128)[:, None]
    for t in range(4):
        m[t] = ((128 * t + kk) <= qq).astype(np.float32)
    return m


def kernel(x, Wq, Wk, Wv, Wo):
    x = np.asarray(x, dtype=np.float32)
    Wq = np.asarray(Wq, dtype=np.float32)
    Wk = np.asarray(Wk, dtype=np.float32)
    Wv = np.asarray(Wv, dtype=np.float32)
    Wo = np.asarray(Wo, dtype=np.float32)

    if "nc" not in _CACHE:
        _CACHE["nc"] = _build()
    nc = _CACHE["nc"]

    masks = _masks_np()
    xT = [np.ascontiguousarray(x[b].T) for b in range(B)]
    in_maps = []
    for c in range(NCORES):
        b, g = c // 4, c % 4
        rows = slice(256 * g, 256 * g + 256)
        in_maps.append(
            {
                "xT": xT[b],
                "wqT": np.ascontiguousarray(Wq[rows].T),
                "wkT": np.ascontiguousarray(Wk[rows].T),
                "wvT": np.ascontiguousarray(Wv[rows].T),
                "woT": np.ascontiguousarray(Wo[:, rows].T),
                "mask": masks,
                "ones": np.ones((128, HPC), dtype=np.float32),
            }
        )

    trace = False
    if os.environ.get("KERNEL_TRACE") == "1":
        try:  # register the NTFF hook if the boot didn't (agent image lacks antenv.axon_hooks)
            from trn_agent_boot.trn_boot import _ntff_profile_via_ctypes
            from antenv.axon_hooks import set_axon_ntff_profile_hook, get_axon_ntff_profile_hook

            if get_axon_ntff_profile_hook() is None:
                set_axon_ntff_profile_hook(
                    _ntff_profile_via_ctypes("/opt/axon/libaxon_pjrt.so")
                )
            trace = True
        except Exception:
            trace = False

    res = run_bass_kernel_spmd(nc, in_maps, core_ids=list(range(NCORES)), trace=trace)
    _CACHE["exec_time_ns"] = res.exec_time_ns
    y = np.zeros((B, S, D), dtype=np.float32)
    for c in range(NCORES):
        y[c // 4] += res.results[c]["y"]
    return y


# revision 8
# speedup vs baseline: 1.0306x; 1.0306x over previous
"""Multi-head self-attention (B=2, S=2048, D=1024, H=16, causal) on 8 NeuronCores.

Sharding: 32 (batch, head) instances -> 4 heads of one batch per core
(cores 0-3: batch 0, cores 4-7: batch 1; core c owns heads 4*(c%4) .. +3).
Wq/Wk/Wv are split by rows (head dims), Wo by columns; each core computes a
partial y[b] = attn_out_heads @ Wo_cols.T and the host sums the 4 partials
per batch at gather time (tensor-parallel reduce).

Per-core kernel (all matmuls float32r, PE-only, no on-device transposes):
  QT[256,2048] = wqT.T @ xT        (head-pair tiles: rows 0-63 / 64-127)
  KT likewise; V[2048,256] natural, augmented with a ones column per head.
  Scores computed transposed, blockwise [k-tile 128, q-chunk 512]:
      S^T = KT_h.T @ QT_h   -- two heads row-packed (contraction d=64 at
      partition bases 0 / 64 -> concurrent PE row groups)
  P = exp(S^T / 8) on ScalarE (psum pairs [128,1024] to amortize overhead),
  causal masking only on diagonal blocks via precomputed 0/1 mask multiply.
  AV: lhsT = [V_h | 1] [k,65], rhs = P [k,512] -> psum [65,512] accumulated
      over k-tiles = unnormalized out^T (rows 0-63) + softmax denominators
      (row 64). Normalize columns via reciprocal + gpsimd partition
      broadcast + DVE multiply, assembling out_headsT [256, 2048].
  y = out_headsT.T @ woT -> [2048, 1024] partial, DMA'd out.
"""
import os
import sys

sys.path.insert(0, "/opt/trn_rl_repo")

import numpy as np

import concourse.bass as bass  # noqa: F401  (import registers engine types)
import concourse.mybir as mybir
from concourse import bacc
from concourse.tile import TileContext
from concourse.bass_utils import run_bass_kernel_spmd

B, S, D = 2, 2048, 1024
H, HD = 16, 64
NCORES = 8
HPC = 4            # heads per core
SC = 512           # q-chunk width
KT_W = 128         # k-tile width
NQC = S // SC      # 4 q-chunks
NKT = S // KT_W    # 16 k-tiles
F32R = mybir.dt.float32r
F32 = mybir.dt.float32
ATTN_SCALE = 1.0 / np.sqrt(HD)

_CACHE = {}


def _build():
    nc = bacc.Bacc("TRN2", target_bir_lowering=False, debug=False, num_devices=NCORES)

    xT_d = nc.declare_dram_parameter("xT", [D, S], F32R, isOutput=False)
    wqT_d = nc.declare_dram_parameter("wqT", [D, 256], F32R, isOutput=False)
    wkT_d = nc.declare_dram_parameter("wkT", [D, 256], F32R, isOutput=False)
    wvT_d = nc.declare_dram_parameter("wvT", [D, 256], F32R, isOutput=False)
    woT_d = nc.declare_dram_parameter("woT", [256, D], F32R, isOutput=False)
    mask_d = nc.declare_dram_parameter("mask", [4, 128, SC], F32R, isOutput=False)
    ones_d = nc.declare_dram_parameter("ones", [128, HPC], F32R, isOutput=False)
    y_d = nc.declare_dram_parameter("y", [S, D], F32, isOutput=True)

    with TileContext(nc) as tc:
        with (
            tc.tile_pool(name="static", bufs=1) as st,
            tc.tile_pool(name="ppool", bufs=2) as ppool,
            tc.tile_pool(name="rbpool", bufs=2) as rbpool,
            tc.tile_pool(name="recpool", bufs=1) as recpool,
            tc.tile_pool(name="psA", bufs=2, space="PSUM") as psA,
            tc.tile_pool(name="psS", bufs=1, space="PSUM") as psS,
            tc.tile_pool(name="psV", bufs=1, space="PSUM") as psV,
        ):
            # ---- input DMAs ----
            xT = [st.tile([128, S], F32R, name=f"xT{k}", tag=f"xT{k}") for k in range(8)]
            for k in range(8):
                nc.sync.dma_start(out=xT[k][:], in_=xT_d[128 * k : 128 * k + 128, :])
            wq = st.tile([128, 2048], F32R, name="wq", tag="wq")
            wk = st.tile([128, 2048], F32R, name="wk", tag="wk")
            wv = st.tile([128, 2048], F32R, name="wv", tag="wv")
            for wt, wd in ((wq, wqT_d), (wk, wkT_d), (wv, wvT_d)):
                for k in range(8):
                    nc.sync.dma_start(
                        out=wt[:, 256 * k : 256 * k + 256],
                        in_=wd[128 * k : 128 * k + 128, :],
                    )
            wo = st.tile([128, 2048], F32R, name="wo", tag="wo")
            for cc in range(2):
                nc.sync.dma_start(
                    out=wo[:, 1024 * cc : 1024 * cc + 1024],
                    in_=woT_d[128 * cc : 128 * cc + 128, :],
                )
            mask = st.tile([128, 4 * SC], F32R, name="mask", tag="mask")
            for t in range(4):
                nc.sync.dma_start(out=mask[:, SC * t : SC * t + SC], in_=mask_d[t])

            # ---- projections ----
            QT = [st.tile([128, S], F32R, name=f"QT{m}", tag=f"QT{m}") for m in range(2)]
            KT = [st.tile([128, S], F32R, name=f"KT{m}", tag=f"KT{m}") for m in range(2)]
            for dst, w in ((QT, wq), (KT, wk)):
                for m in range(2):
                    for n in range(NQC):
                        acc = psA.tile([128, SC], F32, name="acc", tag="acc")
                        for k in range(8):
                            nc.tensor.matmul(
                                acc[:],
                                w[:, 256 * k + 128 * m : 256 * k + 128 * m + 128],
                                xT[k][:, SC * n : SC * n + SC],
                                start=(k == 0),
                                stop=(k == 7),
                            )
                        nc.vector.tensor_copy(dst[m][:, SC * n : SC * n + SC], acc[:])

            va = [
                st.tile([128, 65 * HPC], F32R, name=f"va{i}", tag=f"va{i}")
                for i in range(NKT)
            ]
            for i in range(NKT):
                accv = psA.tile([128, 256], F32, name="accv", tag="acc")
                for k in range(8):
                    nc.tensor.matmul(
                        accv[:],
                        xT[k][:, 128 * i : 128 * i + 128],
                        wv[:, 256 * k : 256 * k + 256],
                        start=(k == 0),
                        stop=(k == 7),
                    )
                for h in range(HPC):
                    nc.vector.tensor_copy(
                        va[i][:, 65 * h : 65 * h + 64], accv[:, 64 * h : 64 * h + 64]
                    )
                ones_ap = va[i].rearrange("p (h c) -> p h c", c=65)[:, :, 64]
                nc.sync.dma_start(out=ones_ap, in_=ones_d[:])

            # ---- attention ----
            outT = [
                st.tile([128, S], F32R, name=f"outT{m}", tag=f"outT{m}") for m in range(2)
            ]
            for hp in range(2):
                for jq in range(NQC):
                    nkt = 4 * jq + 4  # causal: k-tiles 0 .. 4*jq+3
                    av = [
                        psV.tile([65, SC], F32, name=f"av{u}", tag=f"av{u}")
                        for u in range(2)
                    ]
                    for kp in range((nkt + 1) // 2):
                        sp = [
                            psS.tile([128, 1024], F32, name=f"sp{u}", tag=f"sp{u}")
                            for u in range(2)
                        ]
                        kts = [2 * kp, 2 * kp + 1]  # nkt is always even
                        for u, base in enumerate((0, 64)):
                            for j, kt in enumerate(kts):
                                nc.tensor.matmul(
                                    sp[u][:, 512 * j : 512 * j + 512],
                                    KT[hp][base : base + 64, 128 * kt : 128 * kt + 128],
                                    QT[hp][base : base + 64, SC * jq : SC * jq + SC],
                                    start=True,
                                    stop=True,
                                )
                        pt = [
                            ppool.tile([128, 1024], F32R, name=f"pt{u}", tag=f"pt{u}")
                            for u in range(2)
                        ]
                        for u in range(2):
                            nc.scalar.activation(
                                pt[u][:],
                                sp[u][:],
                                mybir.ActivationFunctionType.Exp,
                                scale=float(ATTN_SCALE),
                            )
                        for j, kt in enumerate(kts):
                            t = kt - 4 * jq
                            if t >= 0:  # diagonal block: causal mask
                                for u in range(2):
                                    nc.vector.tensor_mul(
                                        pt[u][:, 512 * j : 512 * j + 512],
                                        pt[u][:, 512 * j : 512 * j + 512],
                                        mask[:, SC * t : SC * t + SC],
                                    )
                        for j, kt in enumerate(kts):
                            for u in range(2):
                                h = 2 * hp + u
                                nc.tensor.matmul(
                                    av[u][:],
                                    va[kt][:, 65 * h : 65 * h + 65],
                                    pt[u][:, 512 * j : 512 * j + 512],
                                    start=(kt == 0),
                                    stop=(kt == nkt - 1),
                                )
                    # normalize: columns /= denominators (row 64)
                    for u in range(2):
                        rec = recpool.tile([1, SC], F32, name="rec", tag="rec")
                        nc.vector.reciprocal(rec[:], av[u][64:65, :])
                        rb = rbpool.tile([64, SC], F32, name="rb", tag="rb")
                        nc.gpsimd.partition_broadcast(rb[:], rec[:])
                        nc.vector.tensor_mul(
                            outT[hp][64 * u : 64 * u + 64, SC * jq : SC * jq + SC],
                            av[u][0:64, :],
                            rb[:],
                        )

            # ---- output projection ----
            for i in range(NKT):  # 16 s-tiles of 128
                for n in range(2):  # dout chunks of 512
                    yp = psA.tile([128, 512], F32, name="yp", tag="acc")
                    for cc in range(2):
                        nc.tensor.matmul(
                            yp[:],
                            outT[cc][:, 128 * i : 128 * i + 128],
                            wo[:, 1024 * cc + 512 * n : 1024 * cc + 512 * n + 512],
                            start=(cc == 0),
                            stop=(cc == 1),
                        )
                    ys = ppool.tile([128, 512], F32, name="ys", tag="pt0")
                    nc.vector.tensor_copy(ys[:], yp[:])
                    nc.sync.dma_start(
                        out=y_d[128 * i : 128 * i + 128, 512 * n : 512 * n + 512],
                        in_=ys[:],
                    )

    nc.compile()
    return nc


def _masks_np():
    m = np.zeros((4, 128, SC), dtype=np.float32)
    qq = np.arange(SC)[None, :]
    kk = np.arange(128)[:, None]
    for t in range(4):
        m[t] = ((128 * t + kk) <= qq).astype(np.float32)
    return m


def kernel(x, Wq, Wk, Wv, Wo):
    x = np.asarray(x, dtype=np.float32)
    Wq = np.asarray(Wq, dtype=np.float32)
    Wk = np.asarray(Wk, dtype=np.float32)
    Wv = np.asarray(Wv, dtype=np.float32)
    Wo = np.asarray(Wo, dtype=np.float32)

    if "nc" not in _CACHE:
        _CACHE["nc"] = _build()
    nc = _CACHE["nc"]

    masks = _masks_np()
    xT = [np.ascontiguousarray(x[b].T) for b in range(B)]
    in_maps = []
    for c in range(NCORES):
        b, g = c // 4, c % 4
        rows = slice(256 * g, 256 * g + 256)
        in_maps.append(
            {
                "xT": xT[b],
                "wqT": np.ascontiguousarray(Wq[rows].T),
                "wkT": np.ascontiguousarray(Wk[rows].T),
                "wvT": np.ascontiguousarray(Wv[rows].T),
                "woT": np.ascontiguousarray(Wo[:, rows].T),
                "mask": masks,
                "ones": np.ones((128, HPC), dtype=np.float32),
            }
        )

    trace = False
    if os.environ.get("KERNEL_TRACE") == "1":
        try:  # register the NTFF hook if the boot didn't (agent image lacks antenv.axon_hooks)
            from trn_agent_boot.trn_boot import _ntff_profile_via_ctypes
            from antenv.axon_hooks import set_axon_ntff_profile_hook, get_axon_ntff_profile_hook

            if get_axon_ntff_profile_hook() is None:
                set_axon_ntff_profile_hook(
                    _ntff_profile_via_ctypes("/opt/axon/libaxon_pjrt.so")
                )
            trace = True
        except Exception:
            trace = False

    res = run_bass_kernel_spmd(nc, in_maps, core_ids=list(range(NCORES)), trace=trace)
    _CACHE["exec_time_ns"] = res.exec_time_ns
    _CACHE["res"] = res
    y = np.zeros((B, S, D), dtype=np.float32)
    for c in range(NCORES):
        y[c // 4] += res.results[c]["y"]
    return y


# revision 10
# speedup vs baseline: 1.3257x; 1.2863x over previous
"""Multi-head self-attention (B=2, S=2048, D=1024, H=16, causal) on 8 NeuronCores.

Sharding: 32 (batch, head) instances -> 4 heads of one batch per core
(cores 0-3: batch 0, cores 4-7: batch 1; core c owns heads 4*(c%4) .. +3).
Wq/Wk/Wv are split by rows (head dims), Wo by columns; each core computes a
partial y[b] = attn_out_heads @ Wo_cols.T and the host sums the 4 partials
per batch at gather time (tensor-parallel reduce).

Per-core kernel. All matmuls fp16 x fp16 -> fp32 psum (fp16 stationary
operands get fast, reorder-hidden LDWEIGHTS; float32r would force a fused
half-rate weight load serialized with every matmul). No on-device transposes:
  QT[256,2048] = wqT.T @ xT        (head-pair tiles: rows 0-63 / 64-127)
  KT likewise; V[2048,256] natural (lhsT = xT chunks), augmented with a
  ones column per head -> va tiles [128, 4*65].
  Scores computed transposed, blockwise [k-tile 128, q-chunk 512]:
      S^T = KT_h.T @ QT_h   -- two heads row-packed (contraction d=64 at
      partition bases 0 / 64 -> concurrent PE row groups). Four blocks
      (2 k-tiles x 2 heads) share one [128,2048] psum tile; one Exp
      (scale=1/8) per tile on ScalarE -> P fp16 in SBUF.
  Causal masking only on diagonal blocks via precomputed 0/1 mask multiply.
  AV: lhsT = [V_h | 1] fp16 [k,65], rhs = P [k,512] -> psum [65,512]
      accumulated over k-tiles = unnormalized out^T (rows 0-63) + softmax
      denominators (row 64). Normalize columns via reciprocal_approx_fast +
      gpsimd partition broadcast + DVE multiply -> out_headsT [256,2048] fp16.
  y = out_headsT.T @ woT -> [2048, 1024] fp32 partial, DMA'd out.
"""
import os
import sys

sys.path.insert(0, "/opt/trn_rl_repo")

import numpy as np

import concourse.bass as bass  # noqa: F401
import concourse.mybir as mybir
from concourse import bacc
from concourse.tile import TileContext
from concourse.bass_utils import run_bass_kernel_spmd

B, S, D = 2, 2048, 1024
H, HD = 16, 64
NCORES = 8
HPC = 4            # heads per core
SC = 512           # q-chunk width
NQC = S // SC      # 4 q-chunks
NKT = S // 128     # 16 k-tiles
F16 = mybir.dt.float16
F32 = mybir.dt.float32
ATTN_SCALE = 1.0 / np.sqrt(HD)

_CACHE = {}


def _build():
    nc = bacc.Bacc("TRN2", target_bir_lowering=False, debug=False, num_devices=NCORES)

    xT_d = nc.declare_dram_parameter("xT", [D, S], F16, isOutput=False)
    wqT_d = nc.declare_dram_parameter("wqT", [D, 256], F16, isOutput=False)
    wkT_d = nc.declare_dram_parameter("wkT", [D, 256], F16, isOutput=False)
    wvT_d = nc.declare_dram_parameter("wvT", [D, 256], F16, isOutput=False)
    woT_d = nc.declare_dram_parameter("woT", [256, D], F16, isOutput=False)
    mask_d = nc.declare_dram_parameter("mask", [4, 128, SC], F16, isOutput=False)
    ones_d = nc.declare_dram_parameter("ones", [128, HPC], F16, isOutput=False)
    y_d = nc.declare_dram_parameter("y", [S, D], F32, isOutput=True)

    with TileContext(nc) as tc:
        with (
            tc.tile_pool(name="static", bufs=1) as st,
            tc.tile_pool(name="ppool", bufs=2) as ppool,
            tc.tile_pool(name="rbpool", bufs=3) as rbpool,
            tc.tile_pool(name="recpool", bufs=2) as recpool,
            tc.tile_pool(name="ystage", bufs=3) as ystage,
            tc.tile_pool(name="psA", bufs=2, space="PSUM") as psA,
            tc.tile_pool(name="psS", bufs=1, space="PSUM") as psS,
            tc.tile_pool(name="psV", bufs=1, space="PSUM") as psV,
        ):
            # ---- input DMAs ----
            xT = [st.tile([128, S], F16, name=f"xT{k}", tag=f"xT{k}") for k in range(8)]
            for k in range(8):
                nc.sync.dma_start(out=xT[k][:], in_=xT_d[128 * k : 128 * k + 128, :])
            wq = st.tile([128, 2048], F16, name="wq", tag="wq")
            wk = st.tile([128, 2048], F16, name="wk", tag="wk")
            wv = st.tile([128, 2048], F16, name="wv", tag="wv")
            for wt, wd in ((wq, wqT_d), (wk, wkT_d), (wv, wvT_d)):
                for k in range(8):
                    nc.sync.dma_start(
                        out=wt[:, 256 * k : 256 * k + 256],
                        in_=wd[128 * k : 128 * k + 128, :],
                    )
            wo = st.tile([128, 2048], F16, name="wo", tag="wo")
            for cc in range(2):
                nc.sync.dma_start(
                    out=wo[:, 1024 * cc : 1024 * cc + 1024],
                    in_=woT_d[128 * cc : 128 * cc + 128, :],
                )
            mask = st.tile([128, 4 * SC], F16, name="mask", tag="mask")
            for t in range(4):
                nc.sync.dma_start(out=mask[:, SC * t : SC * t + SC], in_=mask_d[t])

            # ---- projections ----
            QT = [st.tile([128, S], F16, name=f"QT{m}", tag=f"QT{m}") for m in range(2)]
            KT = [st.tile([128, S], F16, name=f"KT{m}", tag=f"KT{m}") for m in range(2)]
            for dst, w in ((QT, wq), (KT, wk)):
                for m in range(2):
                    for n in range(NQC):
                        acc = psA.tile([128, SC], F32, name="acc", tag="acc")
                        for k in range(8):
                            nc.tensor.matmul(
                                acc[:],
                                w[:, 256 * k + 128 * m : 256 * k + 128 * m + 128],
                                xT[k][:, SC * n : SC * n + SC],
                                start=(k == 0),
                                stop=(k == 7),
                            )
                        nc.vector.tensor_copy(dst[m][:, SC * n : SC * n + SC], acc[:])

            va = [
                st.tile([128, 65 * HPC], F16, name=f"va{i}", tag=f"va{i}")
                for i in range(NKT)
            ]
            for i in range(NKT):
                accv = psA.tile([128, 256], F32, name="accv", tag="acc")
                for k in range(8):
                    nc.tensor.matmul(
                        accv[:],
                        xT[k][:, 128 * i : 128 * i + 128],
                        wv[:, 256 * k : 256 * k + 256],
                        start=(k == 0),
                        stop=(k == 7),
                    )
                for h in range(HPC):
                    nc.vector.tensor_copy(
                        va[i][:, 65 * h : 65 * h + 64], accv[:, 64 * h : 64 * h + 64]
                    )
                ones_ap = va[i].rearrange("p (h c) -> p h c", c=65)[:, :, 64]
                nc.sync.dma_start(out=ones_ap, in_=ones_d[:])

            # ---- attention ----
            outT = [
                st.tile([128, S], F16, name=f"outT{m}", tag=f"outT{m}") for m in range(2)
            ]
            for hp in range(2):
                for jq in range(NQC):
                    nkt = 4 * jq + 4  # causal: k-tiles 0 .. 4*jq+3
                    av = [
                        psV.tile([65, SC], F32, name=f"av{u}", tag=f"av{u}")
                        for u in range(2)
                    ]
                    for kp in range(nkt // 2):
                        # one psum tile: [head u 1024][k-tile j 512]
                        sp = psS.tile([128, 2048], F32, name="sp", tag="sp")
                        kts = (2 * kp, 2 * kp + 1)
                        for u, base in enumerate((0, 64)):
                            for j, kt in enumerate(kts):
                                nc.tensor.matmul(
                                    sp[:, 1024 * u + 512 * j : 1024 * u + 512 * j + 512],
                                    KT[hp][base : base + 64, 128 * kt : 128 * kt + 128],
                                    QT[hp][base : base + 64, SC * jq : SC * jq + SC],
                                    start=True,
                                    stop=True,
                                )
                        pt = ppool.tile([128, 2048], F16, name="pt", tag="pt")
                        nc.scalar.activation(
                            pt[:],
                            sp[:],
                            mybir.ActivationFunctionType.Exp,
                            scale=float(ATTN_SCALE),
                        )
                        for j, kt in enumerate(kts):
                            t = kt - 4 * jq
                            if t >= 0:  # diagonal block: causal mask
                                for u in range(2):
                                    sl = slice(
                                        1024 * u + 512 * j, 1024 * u + 512 * j + 512
                                    )
                                    nc.vector.tensor_mul(
                                        pt[:, sl], pt[:, sl], mask[:, SC * t : SC * t + SC]
                                    )
                        for j, kt in enumerate(kts):
                            for u in range(2):
                                h = 2 * hp + u
                                nc.tensor.matmul(
                                    av[u][:],
                                    va[kt][:, 65 * h : 65 * h + 65],
                                    pt[:, 1024 * u + 512 * j : 1024 * u + 512 * j + 512],
                                    start=(kt == 0),
                                    stop=(kt == nkt - 1),
                                )
                    # normalize: columns /= denominators (row 64)
                    for u in range(2):
                        den = recpool.tile([1, SC], F32, name="den", tag="den")
                        nc.vector.tensor_copy(den[:], av[u][64:65, :])
                        rec = recpool.tile([1, SC], F32, name="rec", tag="rec")
                        nc.vector.reciprocal_approx_fast(rec[:], den[:])
                        rb = rbpool.tile([64, SC], F32, name="rb", tag="rb")
                        nc.gpsimd.partition_broadcast(rb[:], rec[:])
                        nc.vector.tensor_mul(
                            outT[hp][64 * u : 64 * u + 64, SC * jq : SC * jq + SC],
                            av[u][0:64, :],
                            rb[:],
                        )

            # ---- output projection ----
            for i in range(NKT):  # 16 s-tiles of 128
                for n in range(2):  # dout chunks of 512
                    yp = psA.tile([128, 512], F32, name="yp", tag="acc")
                    for cc in range(2):
                        nc.tensor.matmul(
                            yp[:],
                            outT[cc][:, 128 * i : 128 * i + 128],
                            wo[:, 1024 * cc + 512 * n : 1024 * cc + 512 * n + 512],
                            start=(cc == 0),
                            stop=(cc == 1),
                        )
                    ys = ystage.tile([128, 512], F32, name="ys", tag="ys")
                    nc.vector.tensor_copy(ys[:], yp[:])
                    nc.sync.dma_start(
                        out=y_d[128 * i : 128 * i + 128, 512 * n : 512 * n + 512],
                        in_=ys[:],
                    )

    nc.compile()
    return nc


def _masks_np():
    m = np.zeros((4, 128, SC), dtype=np.float16)
    qq = np.arange(SC)[None, :]
    kk = np.arange(128)[:, None]
    for t in range(4):
        m[t] = ((128 * t + kk) <= qq).astype(np.float16)
    return m


def kernel(x, Wq, Wk, Wv, Wo):
    x = np.asarray(x, dtype=np.float32)
    Wq = np.asarray(Wq, dtype=np.float32)
    Wk = np.asarray(Wk, dtype=np.float32)
    Wv = np.asarray(Wv, dtype=np.float32)
    Wo = np.asarray(Wo, dtype=np.float32)

    if "nc" not in _CACHE:
        _CACHE["nc"] = _build()
    nc = _CACHE["nc"]

    masks = _masks_np()
    xT = [np.ascontiguousarray(x[b].T).astype(np.float16) for b in range(B)]
    in_maps = []
    for c in range(NCORES):
        b, g = c // 4, c % 4
        rows = slice(256 * g, 256 * g + 256)
        in_maps.append(
            {
                "xT": xT[b],
                "wqT": np.ascontiguousarray(Wq[rows].T).astype(np.float16),
                "wkT": np.ascontiguousarray(Wk[rows].T).astype(np.float16),
                "wvT": np.ascontiguousarray(Wv[rows].T).astype(np.float16),
                "woT": np.ascontiguousarray(Wo[:, rows].T).astype(np.float16),
                "mask": masks,
                "ones": np.ones((128, HPC), dtype=np.float16),
            }
        )

    trace = False
    if os.environ.get("KERNEL_TRACE") == "1":
        try:
            from trn_agent_boot.trn_boot import _ntff_profile_via_ctypes
            from antenv.axon_hooks import (
                get_axon_ntff_profile_hook,
                set_axon_ntff_profile_hook,
            )

            if get_axon_ntff_profile_hook() is None:
                set_axon_ntff_profile_hook(
                    _ntff_profile_via_ctypes("/opt/axon/libaxon_pjrt.so")
                )
            trace = True
        except Exception:
            trace = False

    res = run_bass_kernel_spmd(nc, in_maps, core_ids=list(range(NCORES)), trace=trace)
    _CACHE["exec_time_ns"] = res.exec_time_ns
    _CACHE["res"] = res
    y = np.zeros((B, S, D), dtype=np.float32)
    for c in range(NCORES):
        y[c // 4] += res.results[c]["y"]
    return y
